# revision 5
# baseline (speedup 1.0000x reference)
"""Depthwise 3x3 CNN combo kernel for TRN2 (8 NeuronCores, data-parallel).

Computes  out = relu(x*a0 + dwconv(x,w1)*a1 + dwconv(x,w2)*a2 + dwconv(x,w3)*a3)
for x [8, 256, 128, 128] f32, by folding everything into a single 9-tap
depthwise conv (conv is linear in the weights; the residual a0*x is the
center tap):  w_eff = a1*w1 + a2*w2 + a3*w3,  w_eff[:,1,1] += a0.

Sharding: batch dim across the 8 cores (one sample per core).

Per-core layout: channels on partitions (2 blocks of 128), image rows
padded to 130 columns (zero cols at 0 and 129) in the free dim, so every
tap (dy,dx) is a constant free-dim offset into the same SBUF tile.

Two MAC engines in parallel, split by output rows (20:12 of 32 8-row
pair units):
  - TensorE: 9 accumulating diag-matmuls per PSUM bank (bf16, 1
    col/cyc), tap-major across 4 banks so weights load once per tap;
    ScalarE applies relu PSUM->SBUF; ScalarE's queue DMAs the pair out.
  - VectorE (16-row blocks): per tap tensor_scalar_mul into a bf16 tmp
    + tensor_tensor add into a bf16 acc.  STT is NOT used: it has no
    fast DVE perf mode (1x only), while TS runs 4x and TT 2x when all
    operands are bf16, unit-stride, 4B-aligned.  dx==1 taps read a
    GpSimd-made unshifted image copy (xB) to stay 4B-aligned.  Relu via
    in-place tensor_scalar_max (4x); outputs ride the sync queue.
GpSimd (otherwise idle; STT/TT are useless-slow on Pool and STT is
V3-illegal) zeroes pads and produces xB.

Input flows in 32-row chunks (17KB/partition contiguous HBM runs) on
the sync queue into an f32 landing tile; ScalarE casts f32->bf16 into
the padded tile.  Output is bf16 (upcast on host) to halve write
traffic.  Input DMAs are emitted one chunk ahead of compute so the
DVE output triggers sharing the sync queue don't stall prefetch.
"""

import numpy as np

import concourse.bacc as bacc
import concourse.mybir as mybir
from concourse import bass_utils
from concourse.tile import TileContext

# Problem constants (hardcoded per contract).
B = 8
C = 256
H = 128
W = 128
NCORES = 8

CB = 2            # channel blocks of 128
P = 128           # partitions
WP = W + 2        # padded row width
MAXR = 34         # max chunk rows + 2 halo rows

TAPS = [(dy, dx) for dy in range(3) for dx in range(3)]
# DVE tap order: aligned dx in {0,2} first; dx==1 last (their source
# copy xB is produced by GpSimd in parallel with the aligned taps).
TAPS_DVE = [(dy, dx) for dx in (0, 2, 1) for dy in range(3)]

F32 = mybir.dt.float32
BF16 = mybir.dt.bfloat16

# Per-chunk schedule: (cb, h0, nrows, blocks); block = ("pe", (jr, ...))
# for a tap-major PSUM group of 8-row pairs, or ("dve", jr, nrows).
_SCHED = [
    (0, 0, 8, [("pe", (0,))]),
    (0, 8, 24, [("dve", 8, 16), ("pe", (0,))]),
    (0, 32, 32, [("dve", 16, 16), ("pe", (0, 8))]),
    (0, 64, 32, [("dve", 16, 16), ("pe", (0, 8))]),
    (0, 96, 32, [("pe", (0, 8)), ("pe", (16, 24))]),
    (1, 0, 32, [("dve", 16, 16), ("pe", (0, 8))]),
    (1, 32, 32, [("pe", (0, 8)), ("pe", (16, 24))]),
    (1, 64, 32, [("dve", 16, 16), ("pe", (0, 8))]),
    (1, 96, 32, [("dve", 16, 16), ("pe", (0, 8))]),
]


def build_tile_kernel(tc, y_ap, x_ap, wdiag_ap, wvec_ap):
    nc = tc.nc
    relu = mybir.ActivationFunctionType.Relu
    copy = mybir.ActivationFunctionType.Copy

    with (
        tc.tile_pool(name="wpool", bufs=1) as wpool,
        tc.tile_pool(name="xcpool", bufs=3) as xcpool,
        tc.tile_pool(name="xppool", bufs=3) as xppool,
        tc.tile_pool(name="xbpool", bufs=2) as xbpool,
        tc.tile_pool(name="psum", bufs=8, space="PSUM") as psum_pool,
        tc.tile_pool(name="opool", bufs=5) as opool,
        tc.tile_pool(name="vacc", bufs=2) as vaccpool,
        tc.tile_pool(name="vtmp", bufs=2) as vtmppool,
    ):
        landing = {}

        def load_chunk(k):
            """Emit the landing DMA for chunk k (sync queue)."""
            if k in landing or k >= len(_SCHED):
                return
            cb_, h0_, hc_, _ = _SCHED[k]
            cs_ = slice(cb_ * P, (cb_ + 1) * P)
            r0_ = 1 if h0_ == 0 else 0
            r1_ = (hc_ + 1) if h0_ + hc_ == H else (hc_ + 2)
            xc_ = xcpool.tile([P, MAXR, W], F32, tag="xc")
            nc.sync.dma_start(
                xc_[:, r0_:r1_, :],
                x_ap[cs_, h0_ - 1 + r0_ : h0_ - 1 + r1_, :],
            )
            landing[k] = xc_

        # First input chunk's DMA goes out before the weight loads so
        # the critical path to the first cast starts immediately.
        load_chunk(0)

        # Per-block diagonal weight matrices for the PE: [k, cb, tap, m].
        # Weight loads ride the ScalarE HW-DGE queue.
        wdiag = wpool.tile([P, CB, 9, P], BF16)
        nc.scalar.dma_start(wdiag[:], wdiag_ap)
        # Per-channel tap scalars for the DVE: [c, cb, tap].
        wvec = wpool.tile([P, CB, 9], F32)
        nc.scalar.dma_start(wvec[:], wvec_ap)

        for k, (cb, h0, hc, blocks) in enumerate(_SCHED):
            cs = slice(cb * P, (cb + 1) * P)
            r0 = 1 if h0 == 0 else 0
            r1 = (hc + 1) if h0 + hc == H else (hc + 2)
            # Emit the next chunk's landing DMA before this chunk's
            # blocks: DVE output triggers share the sync queue, and
            # emitting the load first keeps input prefetch ahead of
            # those compute-dependent waits.
            load_chunk(k)
            load_chunk(k + 1)
            xc = landing.pop(k)

            # Padded bf16 tile: rows 0..hc+1 map to image rows
            # h0-1 .. h0+hc; cols 1..128 hold the image, cols 0/129
            # zero-padded.  Row stride 260B keeps 4B alignment.
            xp = xppool.tile([P, MAXR, WP], BF16, tag="xp")
            nc.gpsimd.memset(xp[:, 0 : hc + 2, 0:1], 0.0)
            nc.gpsimd.memset(xp[:, 0 : hc + 2, W + 1 : W + 2], 0.0)
            if h0 == 0:
                nc.gpsimd.memset(xp[:, 0:1, 1 : W + 1], 0.0)
            if h0 + hc == H:
                nc.gpsimd.memset(xp[:, hc + 1 : hc + 2, 1 : W + 1], 0.0)
            nc.scalar.activation(
                xp[:, r0:r1, 1 : W + 1], xc[:, r0:r1, :], copy
            )

            for blk in blocks:
                if blk[0] == "dve":
                    _, jr, nr = blk
                    # Unshifted image copy for the dx==1 taps (byte
                    # offset stays 4B-aligned); GpSimd makes it while
                    # the DVE runs the 6 aligned taps.
                    xb = xbpool.tile([P, 18, W], BF16, tag="xb")
                    nc.gpsimd.tensor_copy(
                        xb[:, 0 : nr + 2, :], xp[:, jr : jr + nr + 2, 1 : W + 1]
                    )
                    acc = vaccpool.tile([P, 16 * W], BF16, tag="acc")
                    tmp = vtmppool.tile([P, 16 * W], BF16, tag="tmp")
                    accf = acc[:, 0 : nr * W]
                    tmpf = tmp[:, 0 : nr * W]
                    acc3 = accf.rearrange("p (r w) -> p r w", w=W)
                    tmp3 = tmpf.rearrange("p (r w) -> p r w", w=W)
                    for t, (dy, dx) in enumerate(TAPS_DVE):
                        ti = 3 * dy + dx
                        if dx == 1:
                            rhs = xb[:, dy : dy + nr, :]
                        else:
                            rhs = xp[:, jr + dy : jr + dy + nr, dx : dx + W]
                        sc = wvec[:, cb, ti : ti + 1]
                        if t == 0:
                            nc.vector.tensor_scalar_mul(acc3, rhs, sc)
                        else:
                            nc.vector.tensor_scalar_mul(tmp3, rhs, sc)
                            nc.vector.tensor_add(accf, accf, tmpf)
                    nc.vector.tensor_scalar_max(accf, accf, 0.0)
                    # DVE has no DGE queue; its outputs go on sync.
                    nc.sync.dma_start(
                        y_ap[cs, h0 + jr : h0 + jr + nr, :], acc3
                    )
                else:
                    jrs = blk[1]
                    # Tap-major across this group's PE pairs: one
                    # weight load per tap feeds up to 4 matmuls.
                    psums = [
                        psum_pool.tile([P, 4 * W], F32, name="ps", tag="ps")
                        for _ in range(2 * len(jrs))
                    ]
                    for t, (dy, dx) in enumerate(TAPS):
                        for pi, jr in enumerate(jrs):
                            for half in range(2):
                                hjr = jr + 4 * half
                                rhs = xp[:, hjr + dy : hjr + dy + 4, dx : dx + W]
                                nc.tensor.matmul(
                                    psums[2 * pi + half][:],
                                    lhsT=wdiag[:, cb, t, :],
                                    rhs=rhs,
                                    start=(t == 0),
                                    stop=(t == 8),
                                    skip_group_check=True,
                                )
                    for pi, jr in enumerate(jrs):
                        pair_sb = opool.tile([P, 8 * W], BF16, tag="out")
                        for half in range(2):
                            nc.scalar.activation(
                                pair_sb[:, half * 4 * W : (half + 1) * 4 * W],
                                psums[2 * pi + half][:],
                                relu,
                            )
                        nc.scalar.dma_start(
                            y_ap[cs, h0 + jr : h0 + jr + 8, :],
                            pair_sb[:].rearrange("p (r w) -> p r w", w=W),
                        )


def host_weights(a, w1, w2, w3):
    """Fold the 4-way combine into one 9-tap depthwise kernel; build the
    diag-matrix (PE) and per-channel-vector (DVE) forms."""
    a = np.asarray(a, np.float64)
    w_eff = (
        a[1] * np.asarray(w1, np.float64)[:, 0]
        + a[2] * np.asarray(w2, np.float64)[:, 0]
        + a[3] * np.asarray(w3, np.float64)[:, 0]
    )  # [C, 3, 3]
    w_eff[:, 1, 1] += a[0]
    wtap = w_eff.reshape(C, 9).astype(np.float32)

    import ml_dtypes

    wdiag = np.zeros((P, CB, 9, P), ml_dtypes.bfloat16)
    wvec = np.zeros((P, CB, 9), np.float32)
    idx = np.arange(P)
    for cb in range(CB):
        blk = wtap[cb * P : (cb + 1) * P]  # [128, 9]
        for t in range(9):
            wdiag[idx, cb, t, idx] = blk[:, t].astype(ml_dtypes.bfloat16)
        wvec[:, cb, :] = blk
    return wdiag, wvec


_PROGRAM = None


def _get_program():
    global _PROGRAM
    if _PROGRAM is None:
        nc = bacc.Bacc(
            "TRN2", target_bir_lowering=False, debug=False,
            enable_partition_id=False,
        )
        x_t = nc.dram_tensor("x", [C, H, W], F32, kind="ExternalInput")
        wdiag_t = nc.dram_tensor("wdiag", [P, CB, 9, P], BF16, kind="ExternalInput")
        wvec_t = nc.dram_tensor("wvec", [P, CB, 9], F32, kind="ExternalInput")
        y_t = nc.dram_tensor("y", [C, H, W], BF16, kind="ExternalOutput")
        with TileContext(nc) as tc:
            build_tile_kernel(tc, y_t.ap(), x_t.ap(), wdiag_t.ap(), wvec_t.ap())
        nc.compile()
        _PROGRAM = nc
    return _PROGRAM


def kernel(x, a, w1, w2, w3, _trace=False, _trace_kwargs=None):
    x = np.ascontiguousarray(np.asarray(x, np.float32))
    wdiag, wvec = host_weights(a, w1, w2, w3)
    in_maps = [
        {"x": x[i], "wdiag": wdiag, "wvec": wvec} for i in range(NCORES)
    ]
    nc = _get_program()
    res = bass_utils.run_bass_kernel_spmd(
        nc, in_maps, core_ids=list(range(NCORES)), trace=_trace,
        **(_trace_kwargs or {}),
    )
    out = np.stack([np.asarray(r["y"], np.float32) for r in res.results], axis=0)
    if _trace:
        return out, res
    return out


# revision 10
# speedup vs baseline: 1.3897x; 1.3897x over previous
"""Depthwise 3x3 CNN combo kernel for TRN2 (8 NeuronCores, data-parallel).

Computes  out = relu(x*a0 + dwconv(x,w1)*a1 + dwconv(x,w2)*a2 + dwconv(x,w3)*a3)
for x [8, 256, 128, 128] f32, by folding everything into a single 9-tap
depthwise conv (conv is linear in the weights; the residual a0*x is the
center tap):  w_eff = a1*w1 + a2*w2 + a3*w3,  w_eff[:,1,1] += a0.

Sharding: batch dim across the 8 cores (one sample per core).

Per-core layout: channels on partitions (2 blocks of 128), image rows
padded to 130 columns (zero cols at 0 and 129) in the free dim, so every
tap (dy,dx) is a constant free-dim offset into the same SBUF tile.

Two MAC engines in parallel, split by output rows (20:12 of 32 8-row
pair units):
  - TensorE: 9 accumulating diag-matmuls per PSUM bank (bf16, 1
    col/cyc), tap-major across 4 banks so weights load once per tap;
    ScalarE applies relu PSUM->SBUF; ScalarE's queue DMAs the pair out.
  - VectorE (16-row blocks): per tap tensor_scalar_mul into a bf16 tmp
    + tensor_tensor add into a bf16 acc.  STT is NOT used: it has no
    fast DVE perf mode (1x only), while TS runs 4x and TT 2x when all
    operands are bf16, unit-stride, 4B-aligned.  dx==1 taps read a
    GpSimd-made unshifted image copy (xB) to stay 4B-aligned.  Relu via
    in-place tensor_scalar_max (4x); outputs ride the sync queue.
GpSimd (otherwise idle; STT/TT are useless-slow on Pool and STT is
V3-illegal) zeroes pads and produces xB.

Input flows in 32-row chunks (17KB/partition contiguous HBM runs) on
the sync queue into an f32 landing tile; ScalarE casts f32->bf16 into
the padded tile.  Output is bf16 (upcast on host) to halve write
traffic.  Input DMAs are emitted one chunk ahead of compute so the
DVE output triggers sharing the sync queue don't stall prefetch.
"""

import numpy as np

import concourse.bacc as bacc
import concourse.mybir as mybir
from concourse import bass_utils
from concourse.tile import TileContext

# Problem constants (hardcoded per contract).
B = 8
C = 256
H = 128
W = 128
NCORES = 8

CB = 2            # channel blocks of 128
P = 128           # partitions
WP = W + 2        # padded row width
MAXR = 34         # max chunk rows + 2 halo rows

TAPS = [(dy, dx) for dy in range(3) for dx in range(3)]
# DVE tap order: aligned dx in {0,2} first; dx==1 last (their source
# copy xB is produced by GpSimd in parallel with the aligned taps).
TAPS_DVE = [(dy, dx) for dx in (0, 2, 1) for dy in range(3)]

F32 = mybir.dt.float32
BF16 = mybir.dt.bfloat16

# Per-chunk schedule: (cb, h0, nrows, blocks); block = ("pe", (jr, ...))
# for a tap-major PSUM group of 8-row pairs, or ("dve", jr, nrows).
# DVE blocks are spread across chunks so neither engine starves, with
# only a small 8-row block in the final chunk to shorten the tail.
_SCHED = [
    (0, 0, 8, [("pe", (0,))]),
    (0, 8, 24, [("dve", 8, 16), ("pe", (0,))]),
    (0, 32, 32, [("dve", 16, 16), ("pe", (0, 8))]),
    (0, 64, 32, [("dve", 16, 16), ("pe", (0, 8))]),
    (1, 0, 32, [("dve", 16, 16), ("pe", (0, 8))]),
    (0, 96, 32, [("pe", (0, 8)), ("pe", (16, 24))]),
    (1, 32, 32, [("pe", (0, 8)), ("pe", (16, 24))]),
    (1, 64, 32, [("dve", 16, 16), ("pe", (0, 8))]),
    (1, 96, 32, [("dve", 24, 8), ("pe", (0, 8)), ("pe", (16,))]),
]


def build_tile_kernel(tc, y_ap, x_ap, wdiag_ap, wvec_ap):
    nc = tc.nc
    relu = mybir.ActivationFunctionType.Relu
    copy = mybir.ActivationFunctionType.Copy

    with (
        tc.tile_pool(name="wpool", bufs=1) as wpool,
        tc.tile_pool(name="xcpool", bufs=4) as xcpool,
        tc.tile_pool(name="xppool", bufs=3) as xppool,
        tc.tile_pool(name="psum", bufs=8, space="PSUM") as psum_pool,
        tc.tile_pool(name="opool", bufs=5) as opool,
        tc.tile_pool(name="vacc", bufs=2) as vaccpool,
        tc.tile_pool(name="vtmp", bufs=2) as vtmppool,
    ):
        landing = {}

        def load_chunk(k):
            """Emit the landing DMA for chunk k (sync queue)."""
            if k in landing or k >= len(_SCHED):
                return
            cb_, h0_, hc_, _ = _SCHED[k]
            cs_ = slice(cb_ * P, (cb_ + 1) * P)
            r0_ = 1 if h0_ == 0 else 0
            r1_ = (hc_ + 1) if h0_ + hc_ == H else (hc_ + 2)
            xc_ = xcpool.tile([P, MAXR, W], F32, tag="xc")
            nc.sync.dma_start(
                xc_[:, r0_:r1_, :],
                x_ap[cs_, h0_ - 1 + r0_ : h0_ - 1 + r1_, :],
            )
            landing[k] = xc_

        # First input chunk's DMA goes out before the weight loads so
        # the critical path to the first cast starts immediately.
        load_chunk(0)

        # Per-block diagonal weight matrices for the PE: [k, cb, tap, m].
        # Weight loads ride the ScalarE HW-DGE queue.
        wdiag = wpool.tile([P, CB, 9, P], BF16)
        nc.scalar.dma_start(wdiag[:], wdiag_ap)
        # Per-channel tap scalars for the DVE: [c, cb, tap].
        wvec = wpool.tile([P, CB, 9], F32)
        nc.scalar.dma_start(wvec[:], wvec_ap)

        for k, (cb, h0, hc, blocks) in enumerate(_SCHED):
            cs = slice(cb * P, (cb + 1) * P)
            r0 = 1 if h0 == 0 else 0
            r1 = (hc + 1) if h0 + hc == H else (hc + 2)
            # Emit the next chunk's landing DMA before this chunk's
            # blocks: DVE output triggers share the sync queue, and
            # emitting the load first keeps input prefetch ahead of
            # those compute-dependent waits.
            load_chunk(k)
            load_chunk(k + 1)
            load_chunk(k + 2)
            xc = landing.pop(k)

            # Padded bf16 tile: rows 0..hc+1 map to image rows
            # h0-1 .. h0+hc; cols 1..128 hold the image, cols 0/129
            # zero-padded.  Row stride 260B keeps 4B alignment.
            xp = xppool.tile([P, MAXR, WP], BF16, tag="xp")
            nc.gpsimd.memset(xp[:, 0 : hc + 2, 0:1], 0.0)
            nc.gpsimd.memset(xp[:, 0 : hc + 2, W + 1 : W + 2], 0.0)
            if h0 == 0:
                nc.gpsimd.memset(xp[:, 0:1, 1 : W + 1], 0.0)
            if h0 + hc == H:
                nc.gpsimd.memset(xp[:, hc + 1 : hc + 2, 1 : W + 1], 0.0)
            nc.scalar.activation(
                xp[:, r0:r1, 1 : W + 1], xc[:, r0:r1, :], copy
            )

            # Flat 1D view of the padded tile: the DVE's packed perf
            # modes pay a ~58-cycle penalty per access-pattern row, so
            # all DVE operands are single flat runs over the 130-wide
            # layout.  Pad positions compute harmless junk (pad cols
            # are zero) that is skipped by the output DMA.
            xpf = xp[:].rearrange("p r w -> p (r w)")

            for blk in blocks:
                if blk[0] == "dve":
                    _, jr, nr = blk
                    L = nr * WP - 2
                    acc = vaccpool.tile([P, 16 * WP], BF16, tag="acc")
                    tmp = vtmppool.tile([P, 16 * WP], BF16, tag="tmp")
                    accf = acc[:, 0:L]
                    tmpf = tmp[:, 0:L]
                    for t, (dy, dx) in enumerate(TAPS_DVE):
                        ti = 3 * dy + dx
                        s = (jr + dy) * WP + dx
                        rhs = xpf[:, s : s + L]
                        sc = wvec[:, cb, ti : ti + 1]
                        if t == 0:
                            nc.vector.tensor_scalar_mul(accf, rhs, sc)
                        else:
                            nc.vector.tensor_scalar_mul(tmpf, rhs, sc)
                            nc.vector.tensor_add(accf, accf, tmpf)
                    # Relu must NOT be in-place: same-AP in/out creates a
                    # per-element RAW hazard that runs ~11x slower.
                    nc.vector.tensor_scalar_max(tmpf, accf, 0.0)
                    # DVE has no DGE queue; its outputs go on sync.
                    tmp3 = tmp[:].rearrange("p (r w) -> p r w", w=WP)
                    nc.sync.dma_start(
                        y_ap[cs, h0 + jr : h0 + jr + nr, :],
                        tmp3[:, 0:nr, 0:W],
                    )
                else:
                    jrs = blk[1]
                    # Tap-major across this group's PE pairs: one
                    # weight load per tap feeds up to 4 matmuls.
                    psums = [
                        psum_pool.tile([P, 4 * W], F32, name="ps", tag="ps")
                        for _ in range(2 * len(jrs))
                    ]
                    for t, (dy, dx) in enumerate(TAPS):
                        for pi, jr in enumerate(jrs):
                            for half in range(2):
                                hjr = jr + 4 * half
                                rhs = xp[:, hjr + dy : hjr + dy + 4, dx : dx + W]
                                nc.tensor.matmul(
                                    psums[2 * pi + half][:],
                                    lhsT=wdiag[:, cb, t, :],
                                    rhs=rhs,
                                    start=(t == 0),
                                    stop=(t == 8),
                                    skip_group_check=True,
                                )
                    # One staging tile + one output DMA per group: the
                    # ScalarE DMA trigger costs ~600ns, so batch pairs.
                    grp = opool.tile([P, 16 * W], BF16, tag="out")
                    for pi, jr in enumerate(jrs):
                        for half in range(2):
                            nc.scalar.activation(
                                grp[:, (2 * pi + half) * 4 * W
                                    : (2 * pi + half + 1) * 4 * W],
                                psums[2 * pi + half][:],
                                relu,
                            )
                    nrows = 8 * len(jrs)
                    nc.scalar.dma_start(
                        y_ap[cs, h0 + jrs[0] : h0 + jrs[0] + nrows, :],
                        grp[:, 0 : nrows * W].rearrange(
                            "p (r w) -> p r w", w=W
                        ),
                    )


def host_weights(a, w1, w2, w3):
    """Fold the 4-way combine into one 9-tap depthwise kernel; build the
    diag-matrix (PE) and per-channel-vector (DVE) forms."""
    a = np.asarray(a, np.float64)
    w_eff = (
        a[1] * np.asarray(w1, np.float64)[:, 0]
        + a[2] * np.asarray(w2, np.float64)[:, 0]
        + a[3] * np.asarray(w3, np.float64)[:, 0]
    )  # [C, 3, 3]
    w_eff[:, 1, 1] += a[0]
    wtap = w_eff.reshape(C, 9).astype(np.float32)

    import ml_dtypes

    wdiag = np.zeros((P, CB, 9, P), ml_dtypes.bfloat16)
    wvec = np.zeros((P, CB, 9), np.float32)
    idx = np.arange(P)
    for cb in range(CB):
        blk = wtap[cb * P : (cb + 1) * P]  # [128, 9]
        for t in range(9):
            wdiag[idx, cb, t, idx] = blk[:, t].astype(ml_dtypes.bfloat16)
        wvec[:, cb, :] = blk
    return wdiag, wvec


_PROGRAM = None


def _get_program():
    global _PROGRAM
    if _PROGRAM is None:
        nc = bacc.Bacc(
            "TRN2", target_bir_lowering=False, debug=False,
            enable_partition_id=False,
        )
        x_t = nc.dram_tensor("x", [C, H, W], F32, kind="ExternalInput")
        wdiag_t = nc.dram_tensor("wdiag", [P, CB, 9, P], BF16, kind="ExternalInput")
        wvec_t = nc.dram_tensor("wvec", [P, CB, 9], F32, kind="ExternalInput")
        y_t = nc.dram_tensor("y", [C, H, W], BF16, kind="ExternalOutput")
        with TileContext(nc) as tc:
            build_tile_kernel(tc, y_t.ap(), x_t.ap(), wdiag_t.ap(), wvec_t.ap())
        nc.compile()
        _PROGRAM = nc
    return _PROGRAM


def kernel(x, a, w1, w2, w3, _trace=False, _trace_kwargs=None):
    x = np.ascontiguousarray(np.asarray(x, np.float32))
    wdiag, wvec = host_weights(a, w1, w2, w3)
    in_maps = [
        {"x": x[i], "wdiag": wdiag, "wvec": wvec} for i in range(NCORES)
    ]
    nc = _get_program()
    res = bass_utils.run_bass_kernel_spmd(
        nc, in_maps, core_ids=list(range(NCORES)), trace=_trace,
        **(_trace_kwargs or {}),
    )
    out = np.stack([np.asarray(r["y"], np.float32) for r in res.results], axis=0)
    if _trace:
        return out, res
    return out
